# revision 3
# baseline (speedup 1.0000x reference)
"""Segment-mean (word-pooling) kernel for Trainium2, 8 NeuronCores.

Problem: hidden_states [16, 4096, 768] f32, word_ids [16, 4096] i32
(non-decreasing per row, -1 = special token). Output [16, 2048, 768] f32:
mean of each word's subword embeddings; words with no tokens -> 0.

Strategy: pure data parallelism, 2 samples per core. Per sample, the
segment-mean is computed as a banded one-hot matmul on the PE:
  out[w, h] = sum_s onehot[s, w] * (1/count[w]) * x[s, h]
Tokens are processed in 32 k-tiles of 128; since word ids are
non-decreasing, each k-tile only touches a <=128-wide band of words, so
each k-tile contributes 1-2 matmuls into 128-word output windows
accumulated in PSUM. The one-hot (scaled by per-token reciprocal counts,
computed on host) is built on the vector engine with a single fused
is_equal*mult tensor_scalar op per k-tile against an iota ramp.

The SPMD program is identical on all 8 cores; the (k-tile, window)
pair structure is the union over samples, so per-core data that doesn't
touch a scheduled pair just contributes a zero one-hot block.
"""

import numpy as np

B, S, H = 16, 4096, 768
NUM_WORDS = S // 2  # 2048
N_CORES = 8
SPC = B // N_CORES  # samples per core = 2
P = 128
KT = S // P  # 32 k-tiles per sample
NW = NUM_WORDS // P  # 16 output windows per sample
NSPLITS = ((0, 512), (512, 768))  # matmul free-dim splits of H


def _plan(word_ids: np.ndarray):
    """Per-slot union plan. For each slot (0/1) and k-tile t: the window
    span [minwin, maxwin] over that slot's 8 samples; per window j the
    sorted member k-tiles. Returns (spans, members) per slot."""
    plans = []
    for slot in range(SPC):
        wid = word_ids[slot::SPC]  # the 8 samples this slot sees
        minwin = np.full(KT, NW, np.int64)
        maxwin = np.full(KT, -1, np.int64)
        for b in range(wid.shape[0]):
            row = wid[b]
            for t in range(KT):
                w = row[t * P : (t + 1) * P]
                w = w[w >= 0]
                if w.size:
                    minwin[t] = min(minwin[t], w.min() // P)
                    maxwin[t] = max(maxwin[t], w.max() // P)
        members = {j: [] for j in range(NW)}
        spans = []
        for t in range(KT):
            if maxwin[t] < 0:  # no valid token anywhere (can't happen)
                spans.append((0, 0))
                continue
            spans.append((int(minwin[t]), int(maxwin[t])))
            for j in range(int(minwin[t]), int(maxwin[t]) + 1):
                members[j].append(t)
        plans.append((spans, members))
    return plans


def _recip_counts(word_ids: np.ndarray) -> np.ndarray:
    """Per-token 1/count(word) as f32; 0 for special (-1) tokens."""
    r = np.zeros((B, S), np.float32)
    for b in range(B):
        wid = word_ids[b]
        valid = wid >= 0
        counts = np.bincount(wid[valid], minlength=NUM_WORDS)
        r[b, valid] = (1.0 / counts[wid[valid]]).astype(np.float32)
    return r


def _build(plans, reps=1):
    """Build + compile the SPMD Bass program. reps>1 repeats the whole
    body for amortized wall-clock timing."""
    import concourse.bacc as bacc
    import concourse.tile as tile
    from concourse import mybir

    nc = bacc.Bacc(
        "TRN2",
        target_bir_lowering=False,
        debug=False,
        enable_asserts=False,
        num_devices=N_CORES,
    )
    f32 = mybir.dt.float32
    x = nc.dram_tensor("x", [SPC * S, H], f32, kind="ExternalInput").ap()
    widf = nc.dram_tensor("widf", [SPC, P, KT], f32, kind="ExternalInput").ap()
    rcp = nc.dram_tensor("rcp", [SPC, P, KT], f32, kind="ExternalInput").ap()
    y = nc.dram_tensor("y", [SPC * NUM_WORDS, H], f32, kind="ExternalOutput").ap()

    IOTA_W = NUM_WORDS + 2 * P  # ramp long enough for any window pair
    max_span = max(
        (jhi - jlo + 1) for spans, _ in plans for (jlo, jhi) in spans
    )

    with tile.TileContext(nc) as tc:
        with (
            tc.tile_pool(name="const", bufs=1) as const_pool,
            tc.tile_pool(name="xin", bufs=10) as x_pool,
            tc.tile_pool(name="oh", bufs=8) as oh_pool,
            tc.tile_pool(name="ev", bufs=4) as ev_pool,
            tc.tile_pool(name="psum", bufs=3, space="PSUM") as psum_pool,
        ):
            iota_i = const_pool.tile([P, IOTA_W], mybir.dt.int32)
            nc.gpsimd.iota(iota_i[:], pattern=[[1, IOTA_W]], base=0, channel_multiplier=0)
            iota_f = const_pool.tile([P, IOTA_W], f32)
            nc.vector.tensor_copy(out=iota_f[:], in_=iota_i[:])

            for rep in range(reps):
                for slot in range(SPC):
                    spans, members = plans[slot]
                    wid_t = const_pool.tile(
                        [P, KT], f32, name=f"wid_{rep}_{slot}", tag=f"wid{slot}"
                    )
                    nc.sync.dma_start(out=wid_t[:], in_=widf[slot, :, :])
                    rcp_t = const_pool.tile(
                        [P, KT], f32, name=f"rcp_{rep}_{slot}", tag=f"rcp{slot}"
                    )
                    nc.sync.dma_start(out=rcp_t[:], in_=rcp[slot, :, :])

                    x_tiles = {}
                    oh_tiles = {}

                    def get_x(t):
                        if t not in x_tiles:
                            xt = x_pool.tile([P, H], f32, name=f"xt_{rep}_{slot}_{t}", tag="xt")
                            r0 = slot * S + t * P
                            nc.sync.dma_start(out=xt[:], in_=x[r0 : r0 + P, :])
                            x_tiles[t] = xt
                        return x_tiles[t]

                    def get_oh(t):
                        if t not in oh_tiles:
                            jlo, jhi = spans[t]
                            wspan = (jhi - jlo + 1) * P
                            oh = oh_pool.tile(
                                [P, max_span * P],
                                f32,
                                name=f"oh_{rep}_{slot}_{t}",
                                tag="oh",
                            )
                            nc.vector.tensor_scalar(
                                out=oh[:, :wspan],
                                in0=iota_f[:, jlo * P : jlo * P + wspan],
                                scalar1=wid_t[:, t : t + 1],
                                scalar2=rcp_t[:, t : t + 1],
                                op0=mybir.AluOpType.is_equal,
                                op1=mybir.AluOpType.mult,
                            )
                            oh_tiles[t] = oh
                        return oh_tiles[t]

                    for j in range(NW):
                        out_sb = ev_pool.tile(
                            [P, H], f32, name=f"out_{rep}_{slot}_{j}", tag="out"
                        )
                        ks = members[j]
                        if not ks:
                            nc.vector.memset(out_sb[:], 0.0)
                        else:
                            ps = psum_pool.tile(
                                [P, H], f32, name=f"ps_{rep}_{slot}_{j}", tag="ps"
                            )
                            for ki, t in enumerate(ks):
                                xt = get_x(t)
                                oh = get_oh(t)
                                jlo = spans[t][0]
                                off = (j - jlo) * P
                                for lo, hi in NSPLITS:
                                    nc.tensor.matmul(
                                        out=ps[:, lo:hi],
                                        lhsT=oh[:, off : off + P],
                                        rhs=xt[:, lo:hi],
                                        start=(ki == 0),
                                        stop=(ki == len(ks) - 1),
                                    )
                            nc.scalar.copy(out=out_sb[:], in_=ps[:])
                        r0 = slot * NUM_WORDS + j * P
                        nc.sync.dma_start(out=y[r0 : r0 + P, :], in_=out_sb[:])

    nc.compile()
    return nc


def _prep_inputs(hidden_states, word_ids):
    hs = np.ascontiguousarray(np.asarray(hidden_states, dtype=np.float32))
    wid = np.asarray(word_ids, dtype=np.int32)
    assert hs.shape == (B, S, H) and wid.shape == (B, S)
    r = _recip_counts(wid)
    # [B, S] -> [B, P, KT]: element (p, t) = token t*P + p
    widf = np.ascontiguousarray(
        wid.astype(np.float32).reshape(B, KT, P).transpose(0, 2, 1)
    )
    rt = np.ascontiguousarray(r.reshape(B, KT, P).transpose(0, 2, 1))
    in_maps = []
    for c in range(N_CORES):
        sl = slice(c * SPC, (c + 1) * SPC)
        in_maps.append(
            {
                "x": hs[sl].reshape(SPC * S, H),
                "widf": widf[sl],
                "rcp": rt[sl],
            }
        )
    return in_maps


def kernel(hidden_states, word_ids):
    import concourse.bass_utils as bass_utils

    wid = np.asarray(word_ids, dtype=np.int32)
    plans = _plan(wid)
    nc = _build(plans)
    in_maps = _prep_inputs(hidden_states, word_ids)
    res = bass_utils.run_bass_kernel_spmd(nc, in_maps, core_ids=list(range(N_CORES)))
    out = np.empty((B, NUM_WORDS, H), np.float32)
    for c in range(N_CORES):
        yc = res.results[c]["y"]
        for slot in range(SPC):
            out[c * SPC + slot] = yc[slot * NUM_WORDS : (slot + 1) * NUM_WORDS]
    return out
